# revision 66
# baseline (speedup 1.0000x reference)
"""GAT self-attention kernel for Trainium2 (8 NeuronCores, SPMD data-parallel over N).

Math (per graph n):
  h_t = X @ W_t ; q_gate_t = sigmoid(relu(q @ W1_t) @ W2_t)
  s_src_t = (h_t * g1) @ a1 ; s_dst_t = (h_t * g2) @ a2
  score[i,j] = lrelu(s_src_{adj[i,j]-1}[i] + s_dst_{adj[i,j]-1}[j])   (adj>0)
  out = softmax_j(score) @ (h_3 * node_mask)

Device strategy (v2):
  - The 4-way type select over adj is a degree-3 polynomial in (adj-2.5)
    whose per-row coefficients come from folding (gate*a*Vinv) through W_t^T
    into 8 columns appended to the W_3 matmul (one fused matmul produces both
    h_3 and all polynomial coefficients per row).
  - q-gate runs weight-stationary in both layers so every intermediate comes
    out of PSUM already transposed ([feat-part, graph-cols]); no PE transposes
    and no PSUM->SBUF shuttle copies.
  - src-side poly is computed in [i-part, j-free] (gpsimd pre-step + custom
    DVE Horner), PE-transposed into the dst-layout PSUM bank; the dst-side
    poly (custom DVE op) also folds the adj>0 mask by emitting -FLT_MAX.
  - lrelu runs on the scalar engine (Lrelu, same ACT table set as Exp); the
    node mask enters as a per-partition ln(mask) bias on the Exp, and the
    softmax denominator is recovered via 1/mask ones-columns appended to h_3
    in the final matmul.
"""

import numpy as np
from contextlib import ExitStack

import concourse.bass as bass
import concourse.bacc as bacc
import concourse.tile as tile
from concourse import mybir
from concourse import dve_ops
from concourse.dve_spec import (Spec, Src0, Src1, C0, C1, C2, MaxNeg, Zero,
                                One, select)
from concourse.dve_uop import DveOpSpec
from concourse.bass_utils import run_bass_kernel_spmd

NEG_CAP = -3.4028234663852886e38  # -FLT_MAX (MaxNeg)


def _register_dve_op(name, spec):
    """Runtime-register a custom DVE op (fp32-internal fused pipeline)."""
    if name in dve_ops._SUB_OPCODE_FOR_NAME:
        return dve_ops.CUSTOM_DVE_SPECS[name + "_OP"]
    op = dve_ops.DveOp(name, spec, subdim=False, uops_sha={},
                       perf_en={"v3": True, "v4": True})
    dve_ops.OPS.append(op)
    dve_ops.CUSTOM_DVE_SPECS[name] = spec
    dve_ops._SUB_OPCODE_FOR_NAME[name] = (
        max(dve_ops._SUB_OPCODE_FOR_NAME.values()) + 1)
    shas = {}
    for ver in ("v3", "v4"):
        s = DveOpSpec(
            name=name,
            opcode=dve_ops.get_dve_sub_opcode(name),
            uops=dve_ops.lower(spec, ver=ver),
            rd1_en=dve_ops.has_src1(spec),
        ).sha(ver)
        shas[ver] = s
    object.__setattr__(op, "uops_sha", shas)
    dve_ops.CUSTOM_DVE_SPECS[name + "_OP"] = op
    return op


def _register_horner():
    # out = (in0*in1 + s0)*in1 + s1 : cubic tail given t1 = a3*z + a2
    return _register_dve_op("HORNER2A_ANT", Spec(
        body=(Src0 * Src1 + C0) * Src1 + C1,
        reference=lambda in0, in1, s0, s1, imm2: (in0 * in1 + s0) * in1 + s1,
    ))


def _register_hornerm():
    # masked cubic tail: imm2 (a large negative, bf16-safe) where
    # in1 (= adj-2.5) <= -2, i.e. adj == 0
    return _register_dve_op("HORNERM_ANT", Spec(
        body=select(Src1 > (Zero - (One + One)),
                    (Src0 * Src1 + C0) * Src1 + C1, C2),
        reference=lambda in0, in1, s0, s1, imm2: np.where(
            in1 > -2.0, (in0 * in1 + s0) * in1 + s1, imm2),
    ))


f32 = mybir.dt.float32
f32r = mybir.dt.float32r
bf16 = mybir.dt.bfloat16
fp8 = mybir.dt.float8e4
Alu = mybir.AluOpType
Act = mybir.ActivationFunctionType

DEBUG = False
N, E, D, NT = 32, 512, 300, 4
D2 = 2 * D  # 600
NCORES = 8
GPC = N // NCORES  # graphs per core
SLOPE = 0.2

DC3 = [(0, 128), (128, 128), (256, 44)]           # 300 split into <=128 chunks
FL5 = [(0, 128), (128, 128), (256, 128), (384, 128), (512, 88)]  # 600 split
EC4 = [(i * 128, 128) for i in range(4)]          # 512 split into 4 chunks
OC6 = [(s, off, ln) for s in range(2) for (off, ln) in DC3]


def _vinv():
    # centered basis z = adj - 2.5: coeffs a0..a3 of the cubic through
    # (z_t, u_t), z_t in {-1.5,-0.5,0.5,1.5} (well conditioned, exact bf16)
    V = np.array([[((t + 1) - 2.5) ** m for m in range(4)] for t in range(4)],
                 np.float64)
    return np.linalg.inv(V)


def build_nc():
    nc = bacc.Bacc("TRN2", target_bir_lowering=False, debug=False,
                   enable_partition_id=True)

    def din(name, shape, dt=f32):
        return nc.dram_tensor(name, shape, dt, kind="ExternalInput").ap()

    identB = din("identB", [128, 128], bf16)
    identF = din("identF", [128, 128], f32)
    wrm = din("wrm", [4, E], bf16)
    qTp = din("qTp", [128, 3, GPC], fp8)
    w1p = din("w1p", [128, NT * 3, D2], fp8)
    w2p = din("w2p", [128, NT * 5, D2], fp8)
    wTp = din("wTp", [128, NT * 3, D], bf16)
    w3p = din("w3p", [128, 3, D], bf16)
    avep = din("avep", [128, 3, 32])
    lnmp = din("lnmp", [128, GPC * 4])
    rmp = din("rmp", [128, GPC * 4])
    xT = din("xT", [GPC, D, E], bf16)             # input_state[n].T
    adjA = din("adjA", [GPC, E, E], bf16)         # adj - 2.5
    adjB = din("adjB", [GPC, E, E], bf16)         # adj.T - 2.5
    out = nc.dram_tensor("out", [GPC, E, D], bf16, kind="ExternalOutput").ap()
    dbg = None
    if DEBUG:
        dbg = {
            "dbg_sg": nc.dram_tensor("dbg_sg", [NT, 128, 24], bf16,
                                     kind="ExternalOutput").ap(),
            "dbg_cp": nc.dram_tensor("dbg_cp", [128, 3, 32], bf16,
                                     kind="ExternalOutput").ap(),
            "dbg_ck": nc.dram_tensor("dbg_ck", [4, 128, 8], f32,
                                     kind="ExternalOutput").ap(),
            "dbg_hm": nc.dram_tensor("dbg_hm", [4, 128, D + 2], bf16,
                                     kind="ExternalOutput").ap(),
            "dbg_pi": nc.dram_tensor("dbg_pi", [4, 128, E], f32,
                                     kind="ExternalOutput").ap(),
            "dbg_pb": nc.dram_tensor("dbg_pb", [4, 128, E], bf16,
                                     kind="ExternalOutput").ap(),
            "dbg_lr": nc.dram_tensor("dbg_lr", [4, 128, E], bf16,
                                     kind="ExternalOutput").ap(),
            "dbg_eh": nc.dram_tensor("dbg_eh", [4, 128, E], bf16,
                                     kind="ExternalOutput").ap(),
        }

    with tile.TileContext(nc) as tc:
        with ExitStack() as ctx:
            _body(ctx, tc, identB, identF, wrm, qTp, w1p, w2p, wTp, w3p, avep,
                  lnmp, rmp, xT, adjA, adjB, out, dbg)
    nc.compile()
    return nc


def _body(ctx, tc, identB, identF, wrm, qTp, w1p, w2p, wTp, w3p, avep, lnmp,
          rmp, xT, adjA, adjB, out, dbg=None):
    nc = tc.nc
    HORNER = _register_horner()
    HORNERM = _register_hornerm()
    const = ctx.enter_context(tc.tile_pool(name="const", bufs=1))
    wq = ctx.enter_context(tc.tile_pool(name="wq", bufs=1))
    prep = ctx.enter_context(tc.tile_pool(name="prep", bufs=2))
    gzap = ctx.enter_context(tc.tile_pool(name="gzap", bufs=2))
    xpool = ctx.enter_context(tc.tile_pool(name="xpool", bufs=1))
    adjp = ctx.enter_context(tc.tile_pool(name="adjp", bufs=3))
    adjtp = ctx.enter_context(tc.tile_pool(name="adjtp", bufs=3))
    polyp = ctx.enter_context(tc.tile_pool(name="polyp", bufs=3))
    srcp = ctx.enter_context(tc.tile_pool(name="srcp", bufs=12))
    ehp = ctx.enter_context(tc.tile_pool(name="ehp", bufs=12))
    hmp = ctx.enter_context(tc.tile_pool(name="hmp", bufs=16))
    ckp = ctx.enter_context(tc.tile_pool(name="ckp", bufs=12))
    outp = ctx.enter_context(tc.tile_pool(name="outp", bufs=3))
    otp = ctx.enter_context(tc.tile_pool(name="otp", bufs=2))

    # ---- constant + weight loads (sync queue, in consumption order) ----
    QT = const.tile([128, 3, GPC], fp8)
    nc.sync.dma_start(out=QT, in_=qTp)
    IDB = const.tile([128, 128], bf16)
    IDR = const.tile([128, 128], f32r)
    XTs = []
    for n in range(GPC):
        XTn = xpool.tile([128, 3, E], bf16, tag=f"xt_{n}")
        XTs.append(XTn)
    W1T = wq.tile([128, NT * 3, D2], fp8)
    W2T = wq.tile([128, NT * 5, D2], fp8)
    for t in range(NT):
        nc.sync.dma_start(out=W1T[:, 3 * t:3 * t + 3, :],
                          in_=w1p[:, 3 * t:3 * t + 3, :])
        nc.sync.dma_start(out=W2T[:, 5 * t:5 * t + 5, :],
                          in_=w2p[:, 5 * t:5 * t + 5, :])
        if t == 0:
            nc.sync.dma_start(out=IDB, in_=identB)
        if t == 1:
            nc.sync.dma_start(
                out=XTs[0][:, 0:2, :],
                in_=xT[0, 0:256].rearrange("(c p) e -> p c e", p=128))
            nc.sync.dma_start(out=XTs[0][:44, 2, :], in_=xT[0, 256:300])
            nc.sync.dma_start(out=IDR, in_=identF.bitcast(f32r))
    WTT = wq.tile([128, NT * 3, D], bf16)
    nc.sync.dma_start(out=WTT, in_=wTp)
    AVE = const.tile([128, 3, 32], f32)
    nc.sync.dma_start(out=AVE, in_=avep)
    W3CP = const.tile([128, 3, D + 32], bf16)
    nc.sync.dma_start(out=W3CP[:, :, 0:D], in_=w3p)
    LNM = const.tile([128, GPC * 4], f32)
    nc.sync.dma_start(out=LNM, in_=lnmp)
    RM = const.tile([128, GPC * 4], f32)
    nc.sync.dma_start(out=RM, in_=rmp)
    for n in range(1, GPC):
        nc.sync.dma_start(out=XTs[n][:, 0:2, :],
                          in_=xT[n, 0:256].rearrange("(c p) e -> p c e", p=128))
        nc.sync.dma_start(out=XTs[n][:44, 2, :], in_=xT[n, 256:300])

    # ---- q-gate path: weight-stationary, outputs pre-transposed ----
    psq_ctx = tc.tile_pool(name="psq", bufs=2, space="PSUM")
    ps = psq_ctx.__enter__()
    SGs = []
    for t in range(NT):
        ps1 = ps.tile([128, 20], f32, tag="l1")
        for fi, (fo, fl) in enumerate(FL5):
            for ci, (do, dl) in enumerate(DC3):
                nc.tensor.matmul(ps1[:fl, 4 * fi:4 * fi + 4],
                                 W1T[:dl, 3 * t + ci, fo:fo + fl],
                                 QT[:dl, ci, :],
                                 start=(fi == 0 and ci == 0),
                                 stop=(fi == 4 and ci == 2),
                                 skip_group_check=True)
        r1t = prep.tile([128, 20], fp8, tag="r1t")
        nc.scalar.activation(r1t, ps1, Act.Relu)
        ps2 = ps.tile([128, 24], f32, tag="l2")
        for gi, (s, oo, ol) in enumerate(OC6):
            for fi, (fo, fl) in enumerate(FL5):
                nc.tensor.matmul(ps2[:ol, 4 * gi:4 * gi + 4],
                                 W2T[:fl, 5 * t + fi, s * D + oo:s * D + oo + ol],
                                 r1t[:fl, 4 * fi:4 * fi + 4],
                                 start=(gi == 0 and fi == 0),
                                 stop=(gi == 5 and fi == 4),
                                 skip_group_check=True)
        sg = prep.tile([128, 24], bf16, tag=f"sg_{t}")
        nc.scalar.activation(sg, ps2, Act.Sigmoid)
        SGs.append(sg)
        if dbg is not None:
            nc.sync.dma_start(out=dbg["dbg_sg"][t], in_=sg)

    # ---- gza tiles and C' fold; CP lands in W3CP cols 300:332 ----
    psC = ps.tile([32, D], f32, tag="cp")
    mm_i = 0
    for t in range(NT):
        GZ = []
        for ci, (off, ln) in enumerate(DC3):
            g_ = gzap.tile([128, 32], bf16, tag=f"gz_{off}")
            GZ.append(g_)
        for gi, (s, oo, ol) in enumerate(OC6):
            ci = gi % 3
            qg_ap = SGs[t][:ol, 4 * gi:4 * gi + 4]
            qg_rep = bass.AP(tensor=qg_ap.tensor, offset=qg_ap.offset,
                             ap=[qg_ap.ap[0], [0, 4], qg_ap.ap[1]])
            a_ap = AVE[:ol, ci, t * 8 + s * 4:t * 8 + s * 4 + 4]
            a_rep = bass.AP(tensor=a_ap.tensor, offset=a_ap.offset,
                            ap=[a_ap.ap[0], a_ap.ap[1], [0, 4]])
            o_ap = GZ[ci][:ol, s:s + 25]
            o_rep = bass.AP(tensor=o_ap.tensor, offset=o_ap.offset,
                            ap=[o_ap.ap[0], [2, 4], [8, 4]])
            nc.vector.tensor_mul(o_rep, qg_rep, a_rep)
        for ci, (do, dl) in enumerate(DC3):
            nc.tensor.matmul(psC, GZ[ci][:dl, :], WTT[:dl, 3 * t + ci, :],
                             start=(mm_i == 0), stop=(mm_i == NT * 3 - 1))
            mm_i += 1
    SBC2 = prep.tile([32, D], bf16, tag="sbc2")
    nc.scalar.copy(SBC2, psC)
    for ci, (off, ln) in enumerate(DC3):
        pst = ps.tile([128, 32], bf16, tag="pt")
        nc.tensor.matmul(pst[:ln, :], SBC2[:, off:off + ln], IDB[:32, :32],
                         is_transpose=True, start=True, stop=True)
        nc.vector.tensor_copy(W3CP[:ln, ci, D:D + 32], pst[:ln, :])
    if dbg is not None:
        nc.sync.dma_start(out=dbg["dbg_cp"], in_=W3CP[:, :, D:D + 32])
    psq_ctx.__exit__(None, None, None)
    ps = ctx.enter_context(tc.tile_pool(name="ps", bufs=2, space="PSUM"))



    # ---- phase 1 (merged): h3 + poly coefficients in one matmul ----
    HMs, CKs = [], []
    for n in range(GPC):
        HM, CK = [], []
        for ii, (eo, el) in enumerate(EC4):
            psh = ps.tile([128, D + 32], f32, tag="ph")
            for ci, (do, dl) in enumerate(DC3):
                nc.tensor.matmul(psh, XTs[n][:dl, ci, eo:eo + el],
                                 W3CP[:dl, ci, :],
                                 start=(ci == 0), stop=(ci == 2))
            hm = hmp.tile([128, D + 2], bf16, tag="hm")
            nc.scalar.copy(hm[:, 0:D], psh[:, 0:D])
            rm_ap = RM[:, n * 4 + ii:n * 4 + ii + 1]
            rm_rep = bass.AP(tensor=rm_ap.tensor, offset=rm_ap.offset,
                             ap=[rm_ap.ap[0], [0, 2]])
            nc.vector.tensor_copy(hm[:, D:D + 2], rm_rep)
            ck = ckp.tile([128, 8], f32, tag="ck")
            nc.vector.tensor_copy(ck, psh[:, D + 8 * n:D + 8 * n + 8])
            HM.append(hm)
            CK.append(ck)
            if dbg is not None and n == 0:
                nc.sync.dma_start(out=dbg["dbg_ck"][ii], in_=ck)
                nc.sync.dma_start(out=dbg["dbg_hm"][ii], in_=hm)
        HMs.append(HM)
        CKs.append(CK)

    # ---- per graph: src polys (C), dst polys + combine + exp (D),
    #      final matmul + normalize (E) ----
    AJs, ATs, PSIs, EHs = {}, {}, {}, {}

    def emit_C(n):
        AJ4 = adjp.tile([128, 4, E], bf16, tag="aj")
        nc.sync.dma_start(out=AJ4,
                          in_=adjA[n].rearrange("(c p) e -> p c e", p=128))
        AT4 = adjtp.tile([128, 4, E], bf16, tag="at")
        nc.sync.dma_start(out=AT4,
                          in_=adjB[n].rearrange("(c p) e -> p c e", p=128))
        AJs[n], ATs[n] = AJ4, AT4
        PS_I = []
        for ii, (eo, el) in enumerate(EC4):
            aj = AJ4[:, ii, :]
            ck = CKs[n][ii]
            t1 = polyp.tile([128, E], bf16, tag="t1")
            nc.gpsimd.tensor_scalar(t1, aj, ck[:, 6:7], ck[:, 4:5],
                                    Alu.mult, Alu.add)
            pi = srcp.tile([128, E], f32r, tag="pi")
            nc.vector._custom_dve(HORNER, out=pi, in0=t1, in1=aj,
                                  s0=ck[:, 2:3], s1=ck[:, 0:1])
            PS_I.append(pi)
            if dbg is not None and n == 0:
                nc.sync.dma_start(out=dbg["dbg_pi"][ii],
                                  in_=pi.bitcast(f32))
        PSIs[n] = PS_I

    def emit_D(n):
        AT4, CK, PS_I = ATs[n], CKs[n], PSIs[n]
        EH = []
        for jj, (eo, el) in enumerate(EC4):
            at = AT4[:, jj, :]
            ck = CK[jj]
            zt = ps.tile([128, E], f32, tag="zt")
            for ii in range(4):
                nc.tensor.matmul(zt[:, ii * 128:(ii + 1) * 128].bitcast(f32r),
                                 PS_I[ii][:, eo:eo + el], IDR,
                                 is_transpose=True, start=(ii == 0), stop=False,
                                 skip_group_check=True)
            t1b = polyp.tile([128, E], bf16, tag="t1b")
            nc.gpsimd.tensor_scalar(t1b, at, ck[:, 7:8], ck[:, 5:6],
                                    Alu.mult, Alu.add)
            pbm = polyp.tile([128, E], bf16, tag="pbm")
            nc.vector._custom_dve(HORNERM, out=pbm, in0=t1b, in1=at,
                                  s0=ck[:, 3:4], s1=ck[:, 1:2], imm2=-1e30)
            nc.tensor.matmul(zt, IDB, pbm, start=False, stop=True,
                             skip_group_check=True)
            lrt = polyp.tile([128, E], bf16, tag="lrt")
            nc.scalar.activation(lrt, zt, Act.Prelu, alpha=SLOPE)
            eh = ehp.tile([128, E], bf16, tag="eh")
            nc.scalar.activation(eh, lrt, Act.Exp,
                                 bias=LNM[:, n * 4 + jj:n * 4 + jj + 1])
            EH.append(eh)
            if dbg is not None and n == 0:
                nc.sync.dma_start(out=dbg["dbg_pb"][jj], in_=pbm)
                nc.sync.dma_start(out=dbg["dbg_lr"][jj], in_=lrt)
                nc.sync.dma_start(out=dbg["dbg_eh"][jj], in_=eh)
        EHs[n] = EH

    def emit_E(n):
        EH, HM = EHs[n], HMs[n]
        OT = otp.tile([128, 4, D], bf16, tag="ot")
        odst = out[n].rearrange("(c p) d -> p c d", p=128)
        for ii, (eo, el) in enumerate(EC4):
            po = ps.tile([128, D + 2], f32, tag="po")
            for jj in range(4):
                nc.tensor.matmul(po, EH[jj][:, eo:eo + el], HM[jj][:, 0:D + 2],
                                 start=(jj == 0), stop=(jj == 3))
            rc = outp.tile([128, 1], f32, tag="rc")
            nc.vector.reciprocal(rc, po[:, D:D + 1])
            nc.scalar.mul(OT[:, ii, :], po[:, 0:D], rc)
            nc.sync.dma_start(out=odst[:, ii:ii + 1, :],
                              in_=OT[:, ii:ii + 1, :])

    # software-pipelined emission: keep 2 graphs in flight so the scheduler
    # always has ready work for every engine
    emit_C(0)
    emit_C(1)
    emit_D(0)
    emit_C(2)
    emit_D(1)
    emit_E(0)
    emit_C(3)
    emit_D(2)
    emit_E(1)
    emit_D(3)
    emit_E(2)
    emit_E(3)


def _prep_inputs(input_state, adj, node_mask, query_vec, W_type, a_type,
                 qattn_W1, qattn_W2):
    import ml_dtypes
    X = np.asarray(input_state, np.float32)
    A = np.asarray(adj, np.int32)
    NMsk = np.asarray(node_mask, np.float32)
    Q = np.asarray(query_vec, np.float32)
    W = np.ascontiguousarray(np.asarray(W_type, np.float32))
    AV = np.asarray(a_type, np.float32)
    W1 = np.ascontiguousarray(np.asarray(qattn_W1, np.float32))
    W2 = np.ascontiguousarray(np.asarray(qattn_W2, np.float32))

    bf = ml_dtypes.bfloat16
    identB = np.ascontiguousarray(np.eye(128, dtype=np.float32)).astype(bf)
    # weight packs (replicated per core)
    w1p = np.zeros((128, NT * 3, D2), np.float32)
    wTp = np.zeros((128, NT * 3, D), np.float32)
    w3p = np.zeros((128, 3, D), np.float32)
    for t in range(NT):
        for ci, (do, dl) in enumerate(DC3):
            w1p[:dl, 3 * t + ci, :] = W1[t, do:do + dl, :]
            wTp[:dl, 3 * t + ci, :] = W[t, :, do:do + dl].T
    for ci, (do, dl) in enumerate(DC3):
        w3p[:dl, ci, :] = W[NT - 1, do:do + dl, :]
    w2p = np.zeros((128, NT * 5, D2), np.float32)
    for t in range(NT):
        for fi, (fo, fl) in enumerate(FL5):
            w2p[:fl, 5 * t + fi, :] = W2[t, fo:fo + fl, :]
    w1p = w1p.astype(ml_dtypes.float8_e4m3fn)
    w2p = w2p.astype(ml_dtypes.float8_e4m3fn)
    wTp = wTp.astype(bf)
    w3p = w3p.astype(bf)

    Vi = _vinv()  # [k, t]
    # avep[p, ci, t*8+s*4+k] = a[t, s*300 + ci_off + p] * Vinv[k, t]
    ave0 = (AV.reshape(NT, 2, D, 1).astype(np.float64) *
            Vi.T.reshape(NT, 1, 1, 4)).astype(np.float32)
    avep = np.zeros((128, 3, 32), np.float32)
    for ci, (off, ln) in enumerate(DC3):
        for t in range(NT):
            for s in range(2):
                avep[:ln, ci, t * 8 + s * 4:t * 8 + s * 4 + 4] = \
                    ave0[t, s, off:off + ln, :]

    in_maps = []
    for c in range(NCORES):
        sl = slice(c * GPC, (c + 1) * GPC)
        Ac = A[sl]
        Qc = Q[sl]  # [GPC, 300]
        qTp = np.zeros((128, 3, GPC), np.float32)
        for ci, (do, dl) in enumerate(DC3):
            qTp[:dl, ci, :] = Qc[:, do:do + dl].T
        qTp = qTp.astype(ml_dtypes.float8_e4m3fn)
        Mc = np.maximum(NMsk[sl, :, 0], 1e-30)  # [GPC, 512]
        lnmp = np.zeros((128, GPC * 4), np.float32)
        rmp = np.zeros((128, GPC * 4), np.float32)
        for n in range(GPC):
            for jj in range(4):
                mcol = Mc[n, jj * 128:(jj + 1) * 128]
                lnmp[:, n * 4 + jj] = np.log(mcol)
                rmp[:, n * 4 + jj] = 1.0 / mcol
        in_maps.append({
            "identB": identB,
            "identF": np.ascontiguousarray(np.eye(128, dtype=np.float32)),
            "wrm": np.zeros((4, E), np.float32).astype(bf),
            "qTp": qTp,
            "w1p": w1p,
            "w2p": w2p,
            "wTp": wTp,
            "w3p": w3p,
            "avep": avep,
            "lnmp": lnmp,
            "rmp": rmp,
            "xT": np.ascontiguousarray(
                X[sl].transpose(0, 2, 1)).astype(bf),
            "adjA": np.ascontiguousarray(
                (Ac.astype(np.float32) - 2.5)).astype(bf),
            "adjB": np.ascontiguousarray(
                (Ac.transpose(0, 2, 1).astype(np.float32) - 2.5)).astype(bf),
        })
    return in_maps


_NC_CACHE = {}


def kernel(**inputs):
    if "nc" not in _NC_CACHE:
        _NC_CACHE["nc"] = build_nc()
    nc = _NC_CACHE["nc"]
    in_maps = _prep_inputs(**inputs)
    res = run_bass_kernel_spmd(nc, in_maps, list(range(NCORES)))
    outs = [np.asarray(res.results[c]["out"]).astype(np.float32)
            for c in range(NCORES)]
    return np.concatenate(outs, axis=0)
